# revision 78
# baseline (speedup 1.0000x reference)
"""Bidirectional Mamba block on 8 Trainium2 NeuronCores.

Sharding: data-parallel over (direction, batch): core c handles
direction c//4 (0=fwd, 1=bwd) and batch c%4.  The final projection is
linear over the concat([out_fwd, out_bwd]) axis, so each core applies its
direction's half of proj_W and the host sums the two partial outputs
(plus proj_b).  Zero cross-core communication.

Per-core layout is "d-major": tiles are [128 partitions = channel slice,
free = sequence].  The selective-scan recurrence h_t = dA_t*h_{t-1} + dBu_t
runs on the Vector engine's tensor_tensor_scan (prefix scan along the free
dim), once per (state s, channel tile): dA_s = exp(A[:, s] * delta).
"""
import sys

sys.path.insert(0, "/opt/trn_rl_repo")

import numpy as np

import concourse.bass as bass
import concourse.tile as tile
from concourse import mybir
from concourse.bass_utils import run_bass_kernel_spmd
from concourse.vector_clock import ScopedClock

# ---------------------------------------------------------------- shapes
D_MODEL = 768
D_STATE = 16
D_CONV = 4
D_INNER = 1536
DT_RANK = 48
B, L = 4, 1024

P = 128
NDT = D_INNER // P      # 12  channel tiles
NK = D_MODEL // P       # 6   d_model contraction tiles
NE = 2 * D_INNER // P   # 24  in_proj output tiles
NDM = D_MODEL // P      # 6   d_model output tiles
TH = 2                  # two 512-wide t-halves for matmuls
F32 = mybir.dt.float32
AF = mybir.ActivationFunctionType
OP = mybir.AluOpType

N_CORES = 8

BF16 = mybir.dt.bfloat16
F32R = mybir.dt.float32r
SCAN_BF16 = True   # bf16 operands for the scan stage (2x DVE modes)
Y_BF16 = True      # accumulate y in bf16 too (cheaper adds, more error)
MM_DT = "f32r"     # matmul operand dtype: f32 (4 cyc/row), f32r/bf16 (1)
WDT = {"f32": F32, "f32r": F32R, "bf16": BF16}[MM_DT]


def _f32(ap):
    """View a WDT-typed AP as plain fp32 for vector/scalar-engine reads."""
    return ap.bitcast(F32) if MM_DT == "f32r" else ap


MAX_WAITS_PER_INST = 1


class SplitDrainTileContext(tile.TileContext):
    """Walrus in this container rejects >1 sem-wait per instruction; the stock
    kernel-tail drain carries one wait per active processor.  Split them into
    a chain of single-wait SP NOPs."""

    def _drain_and_barrier(self, tick_clock, wait_clock):
        nc = self.nc
        carrier = nc.sync.nop(nofuse=True)
        wait_clock.add_sem_waits(
            carrier.ins, ScopedClock({None: tick_clock.global_clock})
        )
        si = carrier.ins.sync_info
        waits = list(si.on_wait) if si is not None and si.on_wait else []
        if len(waits) > MAX_WAITS_PER_INST:
            carrier.ins.sync_info = mybir.SyncInfo(
                on_wait=waits[:MAX_WAITS_PER_INST], on_update=[]
            )
            rest = waits[MAX_WAITS_PER_INST:]
            for i in range(0, len(rest), MAX_WAITS_PER_INST):
                extra = nc.sync.nop(nofuse=True)
                extra.ins.sync_info = mybir.SyncInfo(
                    on_wait=rest[i : i + MAX_WAITS_PER_INST], on_update=[]
                )
        nc.sync.drain()
        nc.all_engine_barrier()
        assert self.sems is not None
        popped = nc._tile_sem_poison_stack.pop()
        assert popped is self._sem_poison
        nc.clear_and_free_semaphores(list(self.sems.allocated().values()))


def _split_multi_waits(nc):
    """Walrus here accepts at most one sem-wait per instruction.  Tile's
    wait-assignment can attach several (e.g. a matmul waiting on weight DMA +
    rhs producer + PSUM release).  Hoist all but the last wait onto same-
    engine NOPs inserted immediately before the instruction."""
    n = 0
    for fn in nc.m.functions:
        for bb in fn.blocks:
            out = []
            for ins in bb.instructions:
                si = ins.sync_info
                waits = list(si.on_wait) if si is not None and si.on_wait else []
                if len(waits) > 1:
                    for wv in waits[:-1]:
                        nop = mybir.InstNoOp(name=f"wsplit_{n}", ins=[], outs=[])
                        n += 1
                        nop.engine = ins.engine
                        nop.sync_info = mybir.SyncInfo(on_wait=[wv], on_update=[])
                        out.append(nop)
                    ins.sync_info = mybir.SyncInfo(
                        on_wait=[waits[-1]], on_update=list(si.on_update or [])
                    )
                out.append(ins)
            bb.instructions = out
    return n


def _col_block_ap(handle, width, col0, ncols, kcount):
    """AP reading rows [0:128*kcount) x cols [col0:col0+ncols) of a [R, width]
    DRAM tensor as a [128, kcount*ncols] tile (k-blocks side by side)."""
    base = handle[:]
    return bass.AP(
        tensor=base.tensor,
        offset=col0,
        ap=[[width, P], [P * width, kcount], [1, ncols]],
    )


def _bcast_ap(src):
    """AP that reads a [L]-row and broadcasts it across 128 partitions."""
    return bass.AP(
        tensor=src.tensor, offset=src.offset, ap=[[0, P]] + list(src.ap)
    )


# 4 states per scan group; groups are laid out side by side with a 2-column
# spacer so every 1024-slice stays 4B-aligned for the DVE 2x mode.
GRP = 4
GSTRIDE = L + 2            # 1026
GW = GRP * GSTRIDE         # 4104


def _grp_ap(t, n=GRP, goff=0):
    """[P, GW] tile viewed as [P, n, L] starting at group goff (skipping the
    spacer columns)."""
    base = t[:]
    return bass.AP(
        tensor=base.tensor,
        offset=base.offset + goff * GSTRIDE,
        ap=[list(base.ap[0]), [GSTRIDE, n], [1, L]],
    )


def _rep_ap(t, n=GRP):
    """[P, L] tile broadcast along an n-sized middle dim (stride 0)."""
    base = t[:]
    return bass.AP(
        tensor=base.tensor,
        offset=base.offset,
        ap=[list(base.ap[0]), [0, n], [1, L]],
    )


def _bcast_half_ap(src, half):
    """Half-width variant of _bcast_ap: columns [half*512, half*512+512)."""
    return bass.AP(
        tensor=src.tensor, offset=src.offset + half * 512, ap=[[0, P], [1, 512]]
    )


# ---------------------------------------------------------------- program
def _build_program(split_waits=True):
    nc = bass.Bass()

    di = lambda name, shape: nc.dram_tensor(name, shape, F32, kind="ExternalInput")
    dw = lambda name, shape: nc.dram_tensor(name, shape, WDT, kind="ExternalInput")
    db = lambda name, shape: nc.dram_tensor(name, shape, BF16, kind="ExternalInput")
    xT = db("xT", [D_MODEL, L])
    # host pre-tiles the weight col-block gathers into contiguous slabs:
    # row-slab e/m is the [P, k*P] stationary tile (1 descriptor per
    # partition instead of ~768 strided 256B descriptors per load)
    inWT = db("inWT", [NE * P, NK * P])
    convw = di("convw", [P, NDT * D_CONV])   # host pre-tiled [(p), (dt k)]
    convb = di("convb", [P, NDT])
    # per-d diag(D) stationary tiles: the xc*D skip term accumulates into
    # the scan-output PSUM via the PE instead of a DVE STT
    Ddiag = db("Ddiag", [P, NDT * P])
    # bf16: must match the (bf16) xc moving operand — walrus rejects
    # f32r-stationary x bf16-moving matmuls.
    xprojWT = nc.dram_tensor(
        "xprojWT", [D_INNER, DT_RANK + 2 * D_STATE], BF16, kind="ExternalInput"
    )
    dtWT = db("dtWT", [DT_RANK, D_INNER])
    dtb = di("dtb", [P, NDT])
    Aarr = di("Aarr", [P, NDT * D_STATE])    # host pre-tiled -exp(A_log)
    # GT = outW.T @ proj_half.T (host-fused): out_proj and the final
    # projection collapse into part[m2] = sum_dd GT[dd-block, m2-block]·y3_dd
    GT = db("GT", [NDT * P, NDM * P])
    identD = db("ident", [P, P])   # stationary for PE y-accumulation

    # bf16 partials: halves the 3MB output write (the host sums the two
    # direction halves in f32; bf16 rounding adds ~1e-3 rel err)
    part = nc.dram_tensor("part", [D_MODEL, L], BF16, kind="ExternalOutput")

    bc_park = nc.dram_tensor("bc_park", [2 * D_STATE, L],
                             BF16 if SCAN_BF16 else F32)

    with SplitDrainTileContext(nc) as tc:
        from contextlib import ExitStack

        with ExitStack() as g:
            consts = g.enter_context(tc.tile_pool(name="consts", bufs=1))
            psum = g.enter_context(tc.tile_pool(name="psum", bufs=1, space="PSUM"))

            # -------- constants
            convw_sb = consts.tile([P, NDT * D_CONV], F32, name="convw")
            nc.sync.dma_start(out=convw_sb[:], in_=convw[:])
            convb_sb = consts.tile([P, NDT], F32, name="convb")
            nc.sync.dma_start(out=convb_sb[:], in_=convb[:])
            dtb_sb = consts.tile([P, NDT], F32, name="dtb")
            nc.sync.dma_start(out=dtb_sb[:], in_=dtb[:])
            # big, late-use constants go AFTER the xT loads on the gpsimd
            # queue — ~750KB ahead of the first in_proj weight tile would
            # stall the head ~15us
            A_sb = consts.tile([P, NDT * D_STATE], F32, name="A")
            Dd_sb = consts.tile([P, NDT * P], BF16, name="Ddiag")
            ident_sb = consts.tile([P, P], BF16, name="ident")

            e1 = ExitStack()  # [start .. du-end]
            e0 = ExitStack()  # [start .. conv-end]
            e2 = ExitStack()  # [dt_proj .. scan-end]
            e3 = ExitStack()  # [scan .. scan-end]
            e4 = ExitStack()  # [scan .. gate-end]
            e5 = ExitStack()  # [gate .. end]

            # Pool creation order defines the LIFO release stack:
            # e4 (lives to the end) at the bottom, then e5 (projections),
            # e2 (scan phase), and e1/e0 (head, released first) on top.
            # e3 (scan temps) is created after e0/e1 close.
            xcs_pool = e4.enter_context(tc.tile_pool(name="xcs", bufs=1))
            sz_pool = e4.enter_context(tc.tile_pool(name="sz", bufs=2))
            y3pool = e5.enter_context(tc.tile_pool(name="y3", bufs=1))
            mopool = e5.enter_context(tc.tile_pool(name="mo", bufs=1))
            w2pool = e5.enter_context(tc.tile_pool(name="w2", bufs=6))
            otmp = e5.enter_context(tc.tile_pool(name="otmp", bufs=2))
            stream = e5.enter_context(tc.tile_pool(name="stream", bufs=1))
            xdbl_pool = e2.enter_context(tc.tile_pool(name="xdbl", bufs=1))
            wdt_pool = e2.enter_context(tc.tile_pool(name="wdt", bufs=1))
            sptmp_pool = e2.enter_context(tc.tile_pool(name="sptmp", bufs=2))
            # xt stays resident through the scan: the z-gate half of
            # in_proj is emitted inside the d-loop (tensor/scalar slack)
            xt_pool = e2.enter_context(tc.tile_pool(name="xt", bufs=1))
            delta_pool = e2.enter_context(
                tc.tile_pool(name="delta", bufs=2, side="right")
            )
            du_pool = e2.enter_context(tc.tile_pool(name="du", bufs=2, side="right"))
            wpool = e1.enter_context(tc.tile_pool(name="w", bufs=6))
            xi_pool = e0.enter_context(tc.tile_pool(name="xi", bufs=1))
            cacc_pool = e0.enter_context(tc.tile_pool(name="cacc", bufs=2))

            # ---- load xT, spread over 3 DGE queues so the first in_proj
            # tile's operands land ~3x sooner than on one serial queue
            # first in_proj weight tile ahead of everything on sync, then the
            # x loads 2/2/2 over the three DGE queues (xt4/5 on sync: they
            # are consumed last in the k-chain)
            we0 = wpool.tile([P, NK * P], BF16, name="we", tag="we", bufs=3)
            nc.sync.dma_start(out=we0[:], in_=inWT[0:P, :])
            xt_q = [nc.gpsimd, nc.scalar, nc.gpsimd, nc.scalar, nc.sync, None]
            xt_sb = []
            for k in range(NK):
                t = xt_pool.tile([P, L], BF16, name=f"xt{k}", tag=f"xt{k}")
                if xt_q[k] is None:
                    # split the last tile's halves over the two lighter queues
                    nc.gpsimd.dma_start(
                        out=t[:, 0:512], in_=xT[k * P : (k + 1) * P, 0:512]
                    )
                    nc.scalar.dma_start(
                        out=t[:, 512:L], in_=xT[k * P : (k + 1) * P, 512:L]
                    )
                else:
                    xt_q[k].dma_start(out=t[:], in_=xT[k * P : (k + 1) * P, :])
                xt_sb.append(t)
            nc.gpsimd.dma_start(out=A_sb[:], in_=Aarr[:])
            nc.gpsimd.dma_start(out=Dd_sb[:], in_=Ddiag[:])
            nc.gpsimd.dma_start(out=ident_sb[:], in_=identD[:])

            # ---- in_proj: xzT[e,t] = sum_k inWT[k,e]^T x[k,t]
            # bf16: the conv STT chain gets the 2x DVE mode (halves the
            # ~51us serial conv on the head's critical path to scan start)
            xi_sb = [
                xi_pool.tile([P, L + 3], BF16, name=f"xi{d}", tag=f"xi{d}")
                for d in range(NDT)
            ]
            for d in range(NDT):
                nc.vector.memset(xi_sb[d][:, 0:3], 0.0)

            # x-branch of in_proj only — the z half runs inside the scan
            # d-loop where tensor/scalar have slack.
            for e in range(NDT):
                if e == 0:
                    we = we0
                else:
                    we = wpool.tile([P, NK * P], BF16, name="we", tag="we", bufs=3)
                    nc.sync.dma_start(out=we[:], in_=inWT[e * P : (e + 1) * P, :])
                for th in range(TH):
                    ps = psum.tile([P, 512], F32, name="mm", tag="mm", bufs=3)
                    for k in range(NK):
                        nc.tensor.matmul(
                            ps[:],
                            we[:, k * P : (k + 1) * P],
                            xt_sb[k][:, th * 512 : (th + 1) * 512],
                            start=(k == 0),
                            stop=(k == NK - 1),
                        )
                    nc.scalar.copy(
                        xi_sb[e][:, 3 + th * 512 : 3 + (th + 1) * 512], ps[:]
                    )

            # ---- conv + silu -> xc (streamed to DRAM), x_proj accumulates
            # into two PSUM banks held across the d loop
            xdbl_sb = xdbl_pool.tile([P, L], BF16, name="xdbl")
            NR = DT_RANK + 2 * D_STATE  # 80
            psx = [
                psum.tile([P, 512], F32, name=f"mmx{th}", tag=f"mmx{th}")
                for th in range(TH)
            ]
            xc_sb = []
            for d in range(NDT):
                # taps via tensor_scalar (2x mode) + a pairwise add tree —
                # the STT form runs at 1x (no 2x uop for 2-tensor+scalar)
                tk = [
                    cacc_pool.tile([P, L], BF16, name=f"ct{k}", tag=f"ct{k}")
                    for k in range(D_CONV)
                ]
                nc.vector.tensor_scalar(
                    tk[0][:],
                    xi_sb[d][:, 0:L],
                    convw_sb[:, 4 * d : 4 * d + 1],
                    convb_sb[:, d : d + 1],
                    op0=OP.mult,
                    op1=OP.add,
                )
                for k in range(1, D_CONV):
                    nc.vector.tensor_scalar(
                        tk[k][:],
                        xi_sb[d][:, k : k + L],
                        convw_sb[:, 4 * d + k : 4 * d + k + 1],
                        None,
                        op0=OP.mult,
                    )
                acc = cacc_pool.tile([P, L], BF16, name="cacc", tag="cacc")
                nc.vector.tensor_add(tk[0][:], tk[0][:], tk[1][:])
                nc.vector.tensor_add(tk[2][:], tk[2][:], tk[3][:])
                nc.vector.tensor_add(acc[:], tk[0][:], tk[2][:])
                xc_t = xcs_pool.tile([P, L], BF16, name=f"xct{d}", tag=f"xct{d}")
                xc_sb.append(xc_t)
                nc.scalar.activation(xc_t[:], acc[:], AF.Silu)

                wx = wpool.tile([P, NR], BF16, name="wx", tag="wx")
                nc.sync.dma_start(out=wx[:], in_=xprojWT[d * P : (d + 1) * P, :])
                for th in range(TH):
                    nc.tensor.matmul(
                        psx[th][:NR, :],
                        wx[:],
                        xc_t[:, th * 512 : (th + 1) * 512],
                        start=(d == 0),
                        stop=(d == NDT - 1),
                    )
            # park raw B/C rows FIRST (psx-sourced, 32-aligned windows): the
            # bc-park -> broadcast chain is the head's critical tail.  The
            # copies run on the (idle-at-this-point) DVE, not the busy scalar
            # queue, and the B rows park in their own DMA so the first
            # broadcasts start ~1us sooner.
            bcrows = cacc_pool.tile([64, L], BF16, name="bcrows", tag="bcr", bufs=1)
            for th in range(TH):
                nc.scalar.copy(
                    bcrows[0:32, th * 512 : (th + 1) * 512],
                    psx[th][32:64, :],
                )
                nc.scalar.copy(
                    bcrows[32:48, th * 512 : (th + 1) * 512],
                    psx[th][64:80, :],
                )
            nc.sync.dma_start(out=bc_park[0:D_STATE, :], in_=bcrows[16:32, :])
            nc.sync.dma_start(out=bc_park[D_STATE:, :], in_=bcrows[32:48, :])

            # only the dt rows of xdbl are read downstream (dt_proj);
            # rows 48:80 (B/C) now flow exclusively through bcrows.  These
            # copies are same-partition, so the (idle) DVE can do them in
            # parallel with the scalar engine's bcrows copies — the d=0
            # delta chain starts ~2us sooner.
            for th in range(TH):
                nc.vector.tensor_copy(
                    xdbl_sb[:DT_RANK, th * 512 : (th + 1) * 512],
                    psx[th][:DT_RANK, :],
                )

            e0.close()  # free xt/xi/cacc
            e1.close()  # free we/wx

            wdt = wdt_pool.tile([P, D_INNER], BF16, name="wdt")
            nc.sync.dma_start(out=wdt[:DT_RANK, :], in_=dtWT[:])

            # -------- selective scan, d-outer / group-inner (GRP states per
            # group).  B/C broadcasts live in grouped [P, GW] tiles so the
            # du*B prep and the h*C product run as ONE wide DVE op per group
            # (4x fewer per-op overheads).  y(d) accumulates over s in PSUM
            # via identity-matmuls on the (otherwise idle) PE array.
            SDT = BF16 if SCAN_BF16 else F32
            NG = D_STATE // GRP  # 4 groups
            bcpool = e3.enter_context(tc.tile_pool(name="bc", bufs=1))
            ballp = e3.enter_context(tc.tile_pool(name="ball", bufs=1, side="right"))
            hallp = e3.enter_context(tc.tile_pool(name="hall", bufs=1, side="right"))
            hcp = e3.enter_context(tc.tile_pool(name="hCall", bufs=1, side="right"))
            # dA on the LEFT side: the scalar engine's dA writes and the
            # DVE scan's dA reads then hit a different SBUF address range
            # than the right-side b/h/du streams (port-conflict spreading).
            dap = e3.enter_context(tc.tile_pool(name="dAp", bufs=2))

            B_grp, C_grp = [], []
            for g in range(NG):
                B_t = bcpool.tile([P, GW], SDT, name=f"Bg{g}", tag=f"Bg{g}")
                C_t = bcpool.tile([P, GW], SDT, name=f"Cg{g}", tag=f"Cg{g}")
                B_grp.append(B_t)
                C_grp.append(C_t)
            # group-0 B gates the scan start: spread its broadcasts as
            # halves over 3 DGE queues (sync/gpsimd/scalar), s-major so the
            # d=0 per-state prep muls can start as soon as their own state's
            # row lands.  The rest stream on gpsimd with C trailing B.
            q = [nc.sync, nc.gpsimd, nc.scalar]
            for sl in range(GRP):
                dst = B_grp[0][:, sl * GSTRIDE : sl * GSTRIDE + L]
                q[(2 * sl) % 3].dma_start(
                    out=dst[:, 0:512], in_=_bcast_half_ap(bc_park[sl], 0)
                )
                q[(2 * sl + 1) % 3].dma_start(
                    out=dst[:, 512:L], in_=_bcast_half_ap(bc_park[sl], 1)
                )
            # interleave: B(g+1) then C(g) — each group's C lands just after
            # the next group's B, matching the d=0 consumption order
            order = []
            for g in range(1, NG):
                order += [(g, 0)]
                order += [(g - 1, 1)]
            order += [(NG - 1, 1)]
            for g, is_c in order:
                for sl in range(GRP):
                    s = g * GRP + sl
                    src = bc_park[(D_STATE if is_c else 0) + s]
                    dst = (C_grp if is_c else B_grp)[g]
                    nc.gpsimd.dma_start(
                        out=dst[:, sl * GSTRIDE : sl * GSTRIDE + L],
                        in_=_bcast_ap(src),
                    )

            y3_sb = [
                y3pool.tile([P, L], BF16, name=f"y3{d}", tag=f"y3{d}")
                for d in range(NDT)
            ]
            # part accumulator for the staged fused projection
            part_sb = [
                mopool.tile([P, L], BF16, name=f"pa{m}", tag=f"pa{m}")
                for m in range(NDM)
            ]
            held_ps = {}   # (m2,th) -> PSUM tile held open into the tail
            gw_last = {}   # GT row-block tiles of the final stage

            # progressive fused projection: part[m2] += sum_dd G[dd]·y3_dd
            # emitted in stages as y3 tiles appear (PE slack under the scan),
            # with ident-matmul re-injection carrying the bf16 partial
            # between stages.  hold_pairs chains stay open in the (head-only)
            # mmx banks so the post-gate tail is just the G11 matmuls.
            def emit_proj_stage(dds, inject, hold_pairs=0, prefetch=None):
                gw = {}
                for dd in dds:
                    t = w2pool.tile([P, NDM * P], BF16, name="gw", tag="gw",
                                    bufs=4)
                    nc.sync.dma_start(out=t[:], in_=GT[dd * P : (dd + 1) * P, :])
                    gw[dd] = t
                if prefetch is not None:
                    t = w2pool.tile([P, NDM * P], BF16, name="gw", tag="gw",
                                    bufs=4)
                    nc.sync.dma_start(
                        out=t[:], in_=GT[prefetch * P : (prefetch + 1) * P, :]
                    )
                    gw_last[prefetch] = t
                for m2 in range(NDM):
                    for th in range(TH):
                        j = m2 * TH + th
                        hold = j < hold_pairs
                        if hold:
                            ps = psum.tile([P, 512], F32, name="pp",
                                           tag=f"mmx{j}", bufs=1)
                        else:
                            ps = psum.tile([P, 512], F32, name="pp",
                                           tag="mm", bufs=3)
                        sl = slice(th * 512, (th + 1) * 512)
                        if inject:
                            nc.tensor.matmul(
                                ps[:], ident_sb[:], part_sb[m2][:, sl],
                                start=True, stop=False,
                            )
                        for i, dd in enumerate(dds):
                            nc.tensor.matmul(
                                ps[:],
                                gw[dd][:, m2 * P : (m2 + 1) * P],
                                y3_sb[dd][:, sl],
                                start=(not inject and i == 0),
                                stop=(not hold and i == len(dds) - 1),
                            )
                        if hold:
                            held_ps[(m2, th)] = ps
                        else:
                            nc.scalar.copy(part_sb[m2][:, sl], ps[:])

            # gate y3 = (xc*D + y) * silu(z) for iteration dd: the skip term
            # is already in yps (diag-D matmul), the PSUM read runs on the
            # scalar engine.  Emitted one iteration LATE, right after the
            # next d's z-silus, so the Copy sits in the silu table window
            # (no extra ACT table loads) and never stalls the dA stream.
            gate_pend = {}

            def emit_gate_scalar(dd):
                ypsd, szd = gate_pend.pop(dd)
                yc = stream.tile([P, L], BF16, name="yc", tag="yc")
                for th in range(TH):
                    nc.scalar.copy(yc[:, th * 512 : (th + 1) * 512], ypsd[th][:])
                return (dd, yc, szd)

            def emit_gate_dve(p):
                dd, yc, szd = p
                nc.vector.tensor_mul(y3_sb[dd][:], yc[:], szd[:])

            for d in range(NDT):
                # dt_proj + softplus -> delta(d)  (ln(1+exp(z+b)): no
                # softplus in this build's ACT tables)
                delta_d = delta_pool.tile([P, L], BF16, name="dl", tag="dl")
                for th in range(TH):
                    ps = psum.tile([P, 512], F32, name="mm", tag="mm", bufs=3)
                    nc.tensor.matmul(
                        ps[:],
                        wdt[:DT_RANK, d * P : (d + 1) * P],
                        xdbl_sb[:DT_RANK, th * 512 : (th + 1) * 512],
                    )
                    u = sptmp_pool.tile([P, 512], F32, name="spu", tag="spu")
                    from contextlib import nullcontext
                    with tc.high_priority(3000) if d == 0 else nullcontext():
                        nc.scalar.activation(
                            u[:], ps[:], AF.Exp, bias=dtb_sb[:, d : d + 1]
                        )
                        nc.scalar.activation(
                            delta_d[:, th * 512 : (th + 1) * 512],
                            u[:],
                            AF.Ln,
                            bias=1.0,
                        )
                du_d = du_pool.tile([P, L], SDT, name="du", tag="du")
                nc.vector.tensor_mul(du_d[:], delta_d[:], xc_sb[d][:])

                # dA stream runs two slots ahead of the scans so the scalar
                # engine never gates a scan start (dap bufs=2 rotation)
                dA_q = []

                def push_dA(s):
                    from contextlib import nullcontext
                    dA = dap.tile([P, L], SDT, name="dA", tag="dA")
                    col = d * D_STATE + s
                    with tc.high_priority(3000) if (d == 0 and s < 2) else nullcontext():
                        nc.scalar.activation(
                            dA[:], delta_d[:], AF.Exp,
                            scale=A_sb[:, col : col + 1],
                        )
                    dA_q.append(dA)

                push_dA(0)
                push_dA(1)

                # z-gate half of in_proj for this d: tensor/scalar slack
                # under the scan's vector work; sz_d consumed by gate below.
                # For d=0 it is DEFERRED until after the scan groups so the
                # first scan's dt->dA scalar chain isn't stuck behind the
                # silu<->exp table round trips.
                def emit_z(d):
                    wez = w2pool.tile(
                        [P, NK * P], BF16, name="wez", tag="wez", bufs=2
                    )
                    nc.sync.dma_start(
                        out=wez[:], in_=inWT[(NDT + d) * P : (NDT + d + 1) * P, :]
                    )
                    sz_d = sz_pool.tile([P, L], BF16, name="sz", tag="sz")
                    zps = []
                    for th in range(TH):
                        ps = psum.tile([P, 512], F32, name="mm", tag="mm", bufs=3)
                        for k in range(NK):
                            nc.tensor.matmul(
                                ps[:],
                                wez[:, k * P : (k + 1) * P],
                                xt_sb[k][:, th * 512 : (th + 1) * 512],
                                start=(k == 0),
                                stop=(k == NK - 1),
                            )
                        zps.append(ps)
                    for th in range(TH):
                        nc.scalar.activation(
                            sz_d[:, th * 512 : (th + 1) * 512], zps[th][:],
                            AF.Silu,
                        )
                    return sz_d

                sz_d = emit_z(d) if d > 0 else None
                pend_gate = None
                if d == NDT - 1:
                    emit_gate_dve(emit_gate_scalar(d - 1))
                elif d > 0:
                    pend_gate = emit_gate_scalar(d - 1)
                # projection stages on PE slack; the last stage holds the
                # first two chains open (mmx banks) into the tail and
                # prefetches the final GT block
                if d == 5:
                    emit_proj_stage([0, 1, 2, 3], inject=False)
                elif d == 9:
                    emit_proj_stage([4, 5, 6, 7], inject=True)
                elif d == 11:
                    emit_proj_stage([8, 9, 10], inject=True, hold_pairs=2,
                                    prefetch=11)

                yps = [
                    psum.tile([P, 512], F32, name=f"yac{th}", tag=f"yac{th}")
                    for th in range(TH)
                ]
                # skip term xc*D opens the PSUM accumulation (diag(D)
                # stationary), then GRP-state groups: one batched du*B prep,
                # GRP scans over its slices, one batched h*C product.
                for th in range(TH):
                    nc.tensor.matmul(
                        yps[th][:],
                        Dd_sb[:, d * P : (d + 1) * P],
                        xc_sb[d][:, th * 512 : (th + 1) * 512],
                        start=True,
                        stop=False,
                    )
                # the LAST d splits its final 4 states into two groups of 2:
                # the final hC + id-matmul burst on the tail critical path
                # halves
                if d == NDT - 1:
                    groups = [(0, 4), (4, 4), (8, 4), (12, 2), (14, 2)]
                else:
                    groups = [(0, 4), (4, 4), (8, 4), (12, 4)]
                for gi, (s0, gn) in enumerate(groups):
                    g4, goff = s0 // GRP, s0 % GRP
                    glast = gi == len(groups) - 1
                    ball = ballp.tile([P, GW], SDT, name="ball", tag="ball")
                    if d == 0 and gi == 0:
                        # per-state prep: each mul starts as soon as its own
                        # B row's broadcast lands (first-scan latency)
                        for sl in range(gn):
                            nc.vector.tensor_mul(
                                ball[:, sl * GSTRIDE : sl * GSTRIDE + L],
                                du_d[:],
                                B_grp[0][:, sl * GSTRIDE : sl * GSTRIDE + L],
                            )
                    else:
                        nc.vector.tensor_mul(
                            _grp_ap(ball, gn),
                            _rep_ap(du_d, gn),
                            _grp_ap(B_grp[g4], gn, goff),
                        )
                    hall = hallp.tile([P, GW], SDT, name="hall", tag="hall")
                    for sl in range(gn):
                        s = s0 + sl
                        dA = dA_q.pop(0)
                        nc.vector.tensor_tensor_scan(
                            hall[:, sl * GSTRIDE : sl * GSTRIDE + L],
                            dA[:],
                            ball[:, sl * GSTRIDE : sl * GSTRIDE + L],
                            0.0,
                            op0=OP.mult,
                            op1=OP.add,
                        )
                        if s + 2 < D_STATE:
                            push_dA(s + 2)
                    hC = hcp.tile([P, GW], SDT, name="hC", tag="hC")
                    nc.vector.tensor_mul(
                        _grp_ap(hC, gn), _grp_ap(hall, gn),
                        _grp_ap(C_grp[g4], gn, goff),
                    )
                    # th-major on the very last group: the th0 stop fires two
                    # matmuls sooner, releasing the final gate's first half
                    for th in range(TH):
                        for sl in range(gn):
                            off = sl * GSTRIDE + th * 512
                            nc.tensor.matmul(
                                yps[th][:],
                                ident_sb[:],
                                hC[:, off : off + 512],
                                start=False,
                                stop=(glast and sl == gn - 1),
                            )
                    # previous d's gate-mul lands here, after group 0: the
                    # scalar yc copies are certainly done, so DVE never waits
                    if gi == 0 and pend_gate is not None:
                        emit_gate_dve(pend_gate)
                        pend_gate = None
                    if d == 0 and gi == 0:
                        sz_d = emit_z(0)
                gate_pend[d] = (yps, sz_d)

            # final gate reads its PSUM directly on the DVE (two 1x muls):
            # skips the scalar round trip on the tail critical path
            ypsd, szd = gate_pend.pop(NDT - 1)
            for th in range(TH):
                nc.vector.tensor_mul(
                    y3_sb[NDT - 1][:, th * 512 : (th + 1) * 512],
                    ypsd[th][:],
                    szd[:, th * 512 : (th + 1) * 512],
                )

            e3.close()  # free bc/scantmp/hC
            e2.close()  # free xdbl/wdt/sptmp/delta/du

            # ---- projection tail: only the final GT block's matmuls wait
            # on the last gate.  Held chains (mmx banks) just append G11 and
            # go out via a scalar copy; the rest compute G11 alone in PSUM
            # and merge with their bf16 partial on the (idle) DVE, halving
            # the throttled-PE work on the tail critical path.
            out_q = [nc.sync, nc.gpsimd, nc.scalar]
            gw11 = gw_last[NDT - 1]
            for m2 in range(NDM):
                for th in range(TH):
                    sl = slice(th * 512, (th + 1) * 512)
                    ps = held_ps.get((m2, th))
                    o = otmp.tile([P, 512], BF16, name="o", tag="o")
                    if ps is None:
                        ps = psum.tile([P, 512], F32, name="pp", tag="mm",
                                       bufs=3)
                        nc.tensor.matmul(
                            ps[:],
                            gw11[:, m2 * P : (m2 + 1) * P],
                            y3_sb[NDT - 1][:, sl],
                            start=True,
                            stop=True,
                        )
                        nc.vector.tensor_add(o[:], part_sb[m2][:, sl], ps[:])
                    else:
                        nc.tensor.matmul(
                            ps[:],
                            gw11[:, m2 * P : (m2 + 1) * P],
                            y3_sb[NDT - 1][:, sl],
                            start=False,
                            stop=True,
                        )
                        nc.scalar.copy(o[:], ps[:])
                    out_q[(m2 * TH + th) % 3].dma_start(
                        out=part[m2 * P : (m2 + 1) * P, sl], in_=o[:]
                    )

            e5.close()
            e4.close()

    if split_waits:
        _split_multi_waits(nc)
    return nc


_NC_CACHE = None


def _get_program():
    global _NC_CACHE
    if _NC_CACHE is None:
        _NC_CACHE = _build_program()
    return _NC_CACHE


import ml_dtypes

_BF16_NP = ml_dtypes.bfloat16


def _ddiag(Dv):
    """Per-d diag(D) stationary tiles: [P, NDT*P] with block d = diag(D_d)."""
    Dv = np.asarray(Dv, np.float32)
    out = np.zeros((P, NDT * P), np.float32)
    for d in range(NDT):
        out[:, d * P : (d + 1) * P] = np.diag(Dv[d * P : (d + 1) * P])
    return out.astype(_BF16_NP)


def _slab(AT, nout, kcnt):
    """Pre-gather the [P, kcnt*P] stationary col-block tiles of a [kcnt*P,
    nout*P] transposed weight into contiguous row-slabs [nout*P, kcnt*P]."""
    rows = [
        np.concatenate(
            [AT[P * k : P * (k + 1), P * e : P * (e + 1)] for k in range(kcnt)],
            axis=1,
        )
        for e in range(nout)
    ]
    return np.ascontiguousarray(np.concatenate(rows, axis=0))


# ---------------------------------------------------------------- host glue
def _core_inputs(x_b, pfx, inputs):
    """Build the per-core in_map for one (direction, batch)."""
    c = np.ascontiguousarray
    inW = inputs[pfx + "_inW"]
    convw = inputs[pfx + "_convw"]
    convb = inputs[pfx + "_convb"]
    xprojW = inputs[pfx + "_xprojW"]
    dtW = inputs[pfx + "_dtW"]
    dtb = inputs[pfx + "_dtb"]
    Alog = inputs[pfx + "_Alog"]
    Dv = inputs[pfx + "_D"]
    outW = inputs[pfx + "_outW"]

    A = -np.exp(np.asarray(Alog, np.float32))  # (D_INNER, D_STATE)

    def ptile(v, inner):  # (D_INNER, inner) -> (P, NDT*inner)
        v = np.asarray(v, np.float32).reshape(NDT, P, inner)
        return c(v.transpose(1, 0, 2).reshape(P, NDT * inner))

    return {
        "xT": c(np.asarray(x_b, np.float32).T).astype(_BF16_NP),
        "inWT": _slab(np.asarray(inW, np.float32).T, NE, NK).astype(_BF16_NP),
        "convw": ptile(convw, D_CONV),
        "convb": ptile(np.asarray(convb).reshape(-1, 1), 1),
        "xprojWT": c(np.asarray(xprojW, np.float32).T).astype(_BF16_NP),
        "dtWT": c(np.asarray(dtW, np.float32).T).astype(_BF16_NP),
        "dtb": ptile(np.asarray(dtb).reshape(-1, 1), 1),
        "Aarr": ptile(A, D_STATE),
        "Ddiag": _ddiag(Dv),
        "ident": np.eye(P, dtype=np.float32).astype(_BF16_NP),
    }


def _build_in_maps(inputs):
    x = np.asarray(inputs["x"], np.float32)
    projW = np.asarray(inputs["proj_W"], np.float32)
    in_maps = []
    for core in range(N_CORES):
        direction, b = core // B, core % B
        pfx = "fwd" if direction == 0 else "bwd"
        half = projW[:, :D_MODEL] if direction == 0 else projW[:, D_MODEL:]
        m = _core_inputs(x[b] if direction == 0 else x[b, ::-1], pfx, inputs)
        # GT[e, p] = sum_dm outW.T[e, dm] * half.T[dm, p]: out_proj and
        # the final projection fused into one stationary matrix
        outW = np.asarray(inputs[pfx + "_outW"], np.float32)
        m["GT"] = np.ascontiguousarray(outW.T @ half.T).astype(_BF16_NP)
        in_maps.append(m)
    return in_maps


def kernel(**inputs):
    projb = np.asarray(inputs["proj_b"], np.float32)
    in_maps = _build_in_maps(inputs)
    nc = _get_program()
    res = run_bass_kernel_spmd(nc, in_maps, list(range(N_CORES)))

    out = np.empty((B, L, D_MODEL), np.float32)
    for b in range(B):
        pf = np.asarray(res.results[b]["part"], np.float32)      # (D_MODEL, L)
        pb = np.asarray(res.results[B + b]["part"], np.float32)  # flipped time
        out[b] = (pf + pb[:, ::-1]).T + projb[None, :]
    return out

